# revision 9
# baseline (speedup 1.0000x reference)
"""MoELoRA forward on 8 Trainium2 NeuronCores — fp8 split-precision version.

Data-parallel over tokens (512/core). All heavy matmuls in fp8e4 with
DoubleRow perf mode (2 K-chunks of 128 per instruction at 0.5 cyc/row).
Precision: W and x are each split hi+lo into two e4m3 tensors
(x ~= x8 + xl, 32*W^T ~= W8 + Wl); the base product uses three DR passes
x8@W8 + x8@Wl + xl@W8 (the xl@Wl term is ~0.13% and dropped), giving
~bf16-level accuracy at 0.75x bf16's PE cost. The global x32 weight scale
dodges e4m3's subnormal floor (W sigma ~ 1/32) and is divided out in the
PSUM->SBUF output copies. The LoRA path (~5% of output magnitude) runs
pure fp8: gates 32-scaled into exp(scale=1/32), t = x8@(32 U2^T),
tp = e4(t_psum * gate/16), out += tp @ e4(V2) (zero-padded DR).

All fp8 operands live in one big SBUF tile fed by k-pair-granular DMAs
that PACK each x8/xl k-pair with the matching W k-pair, so both sems of
every matmul's operands fire together and the PE (started at the
computed no-bubble point, warmed by junk matmuls) never starves.
Bulk stores ride SWDGE kv_writeback (packed 16-rows-per-descriptor
format -> tiny DMA-engine time, desc-gen overlapped on the Pool
engine); the final token block is split 256/256 across one HWDGE store
and one writeback so the two completion chains overlap.
"""

import numpy as np
import ml_dtypes

_CACHE = {}

B, S, D_IN, D_OUT, E, R = 4, 1024, 1024, 1024, 8, 16
N_CORES = 8
N_TOK = B * S
TOK = N_TOK // N_CORES        # 512 tokens/core
ER = E * R                    # 128
HB = D_IN // 128              # 8 k-chunks
HP = HB // 2                  # 4 k-pairs
NB = TOK // 128               # 4 token blocks
SW = 32.0                     # global weight scale

BF16 = ml_dtypes.bfloat16
E4 = ml_dtypes.float8_e4m3

# big-tile column layout (fp8 bytes per partition), grouped so each DMA
# segment is one contiguous span: per k-pair [x8|w8c0] and [xl|wlc0]
# pairs, then w8c1(+v28), wlc1, with gw8/u28 riding the last x8 segment
OFF_X8 = [0, 4096, 8192, 12288]
OFF_W0 = [1024, 5120, 9216, 13312]
OFF_XL = [2048, 6144, 10240, 15488]
OFF_L0 = [3072, 7168, 11264, 16512]
OFF_GW = 14336
OFF_U2 = 14464
OFF_W1 = [17536, 18560, 21632, 22656]
OFF_V2 = 19584          # 1024 data + 1024 zeros (zeros memset)
OFF_L1 = [23680, 24704, 25728, 26752]
BIG_COLS = 27776

# DMA segments: (name, [list of (offset, cols)]) in stream order
SEGS = [
    ("P0", [(OFF_X8[0], 1024), (OFF_W0[0], 1024)]),
    ("Q0", [(OFF_XL[0], 1024), (OFF_L0[0], 1024)]),
    ("P1", [(OFF_X8[1], 1024), (OFF_W0[1], 1024)]),
    ("Q1", [(OFF_XL[1], 1024), (OFF_L0[1], 1024)]),
    ("P2", [(OFF_X8[2], 1024), (OFF_W0[2], 1024)]),
    ("Q2", [(OFF_XL[2], 1024), (OFF_L0[2], 1024)]),
    ("P3", [(OFF_X8[3], 1024), (OFF_W0[3], 1024), (OFF_GW, 128), (OFF_U2, 1024)]),
    ("Q3", [(OFF_XL[3], 1024), (OFF_L0[3], 1024)]),
    ("W1a", [(OFF_W1[0], 1024), (OFF_W1[1], 1024), (OFF_V2, 1024)]),
    ("W1b", [(OFF_W1[2], 1024), (OFF_W1[3], 1024)]),
    ("L1a", [(OFF_L1[0], 1024), (OFF_L1[1], 1024)]),
    ("L1b", [(OFF_L1[2], 1024), (OFF_L1[3], 1024)]),
]


def _build():
    import concourse.tile as tile
    import concourse.bass_isa as bass_isa
    from concourse import bacc, mybir
    from contextlib import ExitStack

    f32 = mybir.dt.float32
    bf16 = mybir.dt.bfloat16
    i32 = mybir.dt.int32
    fp8 = mybir.dt.float8e4
    DR = mybir.MatmulPerfMode.DoubleRow
    Exp = mybir.ActivationFunctionType.Exp
    Copy = mybir.ActivationFunctionType.Copy

    nc = bacc.Bacc("TRN2", target_bir_lowering=False, debug=False,
                   num_devices=N_CORES)
    seg_d = {name: nc.dram_tensor(name, [128, sum(c for _, c in chunks)], fp8,
                                  kind="ExternalInput").ap()
             for name, chunks in SEGS}
    sel_d = nc.dram_tensor("sel", [E, 2 * ER], fp8, kind="ExternalInput").ap()
    idx_d = nc.dram_tensor("idx", [128, 8], i32, kind="ExternalInput").ap()
    # per-store output tensors (separate so Tile's conservative WAW
    # tracking can't chain one writeback onto the previous store's DMA)
    outA_d = nc.dram_tensor("outA", [TOK, 512], bf16, kind="ExternalOutput").ap()
    outB_d = [nc.dram_tensor(f"outB{i}", [128, 512], bf16, kind="ExternalOutput").ap()
              for i in range(NB - 1)]
    outF0_d = nc.dram_tensor("outF0", [128, 256], bf16, kind="ExternalOutput").ap()
    outF1_d = nc.dram_tensor("outF1", [128, 256], bf16, kind="ExternalOutput").ap()

    with tile.TileContext(nc) as tc, ExitStack() as ctx:
        sb = ctx.enter_context(tc.tile_pool(name="sb", bufs=1))
        ps = ctx.enter_context(tc.tile_pool(name="ps", bufs=8, space="PSUM"))
        osb = ctx.enter_context(tc.tile_pool(name="osb", bufs=1))

        big = sb.tile([128, BIG_COLS], fp8, tag="big")
        selz = sb.tile([8, 2, ER], fp8, tag="selz")
        g8t = sb.tile([8, 2, TOK], fp8, tag="g8t")
        tpt = sb.tile([128, 2, TOK], fp8, tag="tpt")
        e_sb = sb.tile([8, TOK], f32, tag="e_sb")
        se_sb = sb.tile([8, TOK], f32, tag="se_sb")
        rec_sb = sb.tile([8, TOK], f32, tag="rec_sb")
        gp_sb = sb.tile([128, TOK], f32, tag="gp_sb")
        junk_sb = sb.tile([1, 512], bf16, tag="junk_sb")
        idx = sb.tile([128, 8], i32, tag="idx")
        oA = osb.tile([128, NB, 512], bf16, tag="oA")
        oB = [osb.tile([128, 1, 512], bf16, tag=f"oB{i}", name=f"oB{i}")
              for i in range(NB - 1)]
        oF0 = osb.tile([128, 256], bf16, tag="oF0")
        oF1 = osb.tile([128, 1, 256], bf16, tag="oF1")

        # operand views into the big tile
        def kv(off, cols=1024):
            return big[:, off:off + cols].rearrange("p (c n) -> p c n", c=2)

        x8v = [kv(o) for o in OFF_X8]          # [128, 2, 512] per k-pair
        xlv = [kv(o) for o in OFF_XL]
        w0v = [kv(o) for o in OFF_W0]          # [128, 2, 512] cols 0-511
        l0v = [kv(o) for o in OFF_L0]
        w1v = [kv(o) for o in OFF_W1]          # cols 512-1023
        l1v = [kv(o) for o in OFF_L1]
        gwv = big[:, OFF_GW:OFF_GW + 128].rearrange("p (c n) -> p c n", c=8)
        u2v = big[:, OFF_U2:OFF_U2 + 1024].rearrange("p (c n) -> p c n", c=8)
        v2v = big[:, OFF_V2:OFF_V2 + 2048].rearrange("p (c n) -> p c n", c=2)

        # memsets off the Pool engine (it runs the store desc-gens):
        # junk operand, the v28 zero half, g8t/tpt DR pair slots
        nc.vector.memset(junk_sb[:], 0.0)
        nc.vector.memset(big[:, OFF_V2 + 1024:OFF_V2 + 2048], 0.0)
        nc.vector.memset(g8t[:], 0.0)
        nc.vector.memset(tpt[:], 0.0)

        # ---- input DMAs (SP ring), k-pair-packed consumption order ----
        nc.sync.dma_start(selz[:], sel_d.rearrange("e (s r) -> e s r", s=2))
        for name, chunks in SEGS:
            off0 = chunks[0][0]
            total = sum(c for _, c in chunks)
            contig = all(chunks[i + 1][0] == chunks[i][0] + chunks[i][1]
                         for i in range(len(chunks) - 1))
            assert contig, f"{name} segments must be contiguous in big"
            nc.sync.dma_start(big[:, off0:off0 + total], seg_d[name][:])
        nc.sync.dma_start(idx[:], idx_d[:])

        # ---- stores ride SWDGE kv_writeback (plain, Tile-managed sems) ----
        def wb_store(dst, src_ap, nblk):
            nc.gpsimd.kv_writeback(
                dst.rearrange("(b p) (d n) -> b p d n", p=128, d=1),
                src_ap.rearrange("p (d b) n -> p d b n", d=1),
                idx[:, 0:nblk])

        # ---- PSUM banks ----
        misc = ps.tile([128, 512], f32, tag="ps", name="misc")
        t_ps = ps.tile([128, TOK], f32, tag="ps", name="t")
        accA = [ps.tile([128, 512], f32, tag="ps", name=f"accA{i}") for i in range(NB)]
        junk2 = ps.tile([1, 448], f32, tag="ps", name="junk2")

        # PE warmup junk sized to end at the computed no-bubble start point
        N_WARM = 8
        nc.tensor.matmul(misc[0:1, 0:64], junk_sb[0:1, 0:1], junk_sb[:, 0:64],
                         start=True, stop=False, skip_group_check=True)
        for w in range(N_WARM):
            nc.tensor.matmul(misc[0:1, 0:448], junk_sb[0:1, 0:1], junk_sb[:, 0:448],
                             start=False, stop=(w == N_WARM - 1),
                             skip_group_check=True)

        def mm(acc_ap, xview, wview, tok0, col_lo=0, col_hi=512,
               start=False, stop=False):
            nc.tensor.matmul(acc_ap,
                             xview[:, :, tok0:tok0 + 128],
                             wview[:, :, col_lo:col_hi],
                             start=start, stop=stop,
                             perf_mode=DR, skip_group_check=True)

        def lora_mm(acc_ap, nb, oc, lo=0, hi=512, start=False, stop=False):
            nc.tensor.matmul(acc_ap,
                             tpt[:, :, nb * 128:(nb + 1) * 128],
                             v2v[:, :, oc * 512 + lo:oc * 512 + hi],
                             start=start, stop=stop,
                             perf_mode=DR, skip_group_check=True)

        def dve_copy(dst, src):
            nc.vector.tensor_scalar(dst, src[:], 1.0 / SW, None,
                                    mybir.AluOpType.mult)

        def act_copy(dst, src):
            nc.scalar.activation(dst, src[:], Copy, scale=1.0 / SW)

        # ---- phase A (oc=0), k-pair-triple order tracking the stream:
        # per kp: x8@W8 (+1 gate instr), xl@W8, x8@Wl ----
        for kp in range(HP):
            for nb in range(NB):
                mm(accA[nb][:], x8v[kp], w0v[kp], nb * 128, start=(kp == 0))
            nc.tensor.matmul(misc[0:8, :], gwv[:, 2 * kp:2 * kp + 2, 0:E],
                             x8v[kp][:, :, :],
                             start=(kp == 0), stop=(kp == HP - 1),
                             perf_mode=DR, skip_group_check=True)
            if kp == HP - 1:
                # softmax chain: exp(gl/32) -> sum -> 1/sum -> g8 (fp8)
                nc.scalar.activation(e_sb[:], misc[0:8, :], Exp, scale=1.0 / SW)
                nc.gpsimd.partition_all_reduce(se_sb[:], e_sb[:], channels=8,
                                               reduce_op=bass_isa.ReduceOp.add)
                nc.vector.reciprocal(rec_sb[:], se_sb[:])
                with nc.allow_low_precision(reason="fp8 gate"):
                    nc.vector.tensor_tensor(g8t[:, 0, :], e_sb[:], rec_sb[:],
                                            mybir.AluOpType.mult)
            for nb in range(NB):
                mm(accA[nb][:], xlv[kp], w0v[kp], nb * 128)
            for nb in range(NB):
                mm(accA[nb][:], x8v[kp], l0v[kp], nb * 128)

        # t = U2^T x8, gate expand gp, then tp = t*gp (DVE) while the PE
        # runs the head passes of B0/B1
        for hp in range(HP):
            nc.tensor.matmul(t_ps[:], u2v[:, 2 * hp:2 * hp + 2, :],
                             x8v[hp][:, :, :],
                             start=(hp == 0), stop=(hp == HP - 1),
                             perf_mode=DR, skip_group_check=True)
        gp_ps = ps.tile([128, TOK], f32, tag="ps", name="gp")
        nc.tensor.matmul(gp_ps[:], selz[:], g8t[:], start=True, stop=True,
                         perf_mode=DR, skip_group_check=True)
        nc.scalar.copy(gp_sb[:], gp_ps[:])
        with nc.allow_low_precision(reason="fp8 tp"):
            nc.vector.tensor_tensor(tpt[:, 0, :], t_ps[:], gp_sb[:],
                                    mybir.AluOpType.mult)

        oc = 1
        accB = [ps.tile([128, 512], f32, tag="ps", name=f"accB{i}")
                for i in range(NB - 1)]
        for nb in (0, 1):   # heads: w8-c1 passes fill the tp-chain latency
            for kp in range(HP):
                mm(accB[nb][:], x8v[kp], w1v[kp], nb * 128, start=(kp == 0))
            for kp in range(HP):
                mm(accB[nb][:], xlv[kp], w1v[kp], nb * 128)

        # phase A lora + copies + store
        for nb in range(NB):
            lora_mm(accA[nb][:], nb, 0, stop=True)
        with nc.allow_low_precision(reason="bf16 output"):
            dve_copy(oA[:, 0, :], accA[0])
            act_copy(oA[:, 1, :], accA[1])
            dve_copy(oA[:, 2, :], accA[2])
            act_copy(oA[:, 3, :], accA[3])
        wb_store(outA_d, oA[:], NB)

        # B0/B1 tails: wl-c1 passes + lora, then store
        for nb in (0, 1):
            for kp in range(HP):
                mm(accB[nb][:], x8v[kp], l1v[kp], nb * 128)
            lora_mm(accB[nb][:], nb, 1, stop=True)
            with nc.allow_low_precision(reason="bf16 output"):
                if nb == 0:
                    dve_copy(oB[nb][:, 0, :], accB[nb])
                else:
                    act_copy(oB[nb][:, 0, :], accB[nb])
            wb_store(outB_d[nb], oB[nb][:], 1)

        # B2 block-major
        nb = 2
        lora_mm(accB[nb][:], nb, 1, start=True)
        for kp in range(HP):
            mm(accB[nb][:], x8v[kp], w1v[kp], nb * 128)
        for kp in range(HP):
            mm(accB[nb][:], xlv[kp], w1v[kp], nb * 128)
        for kp in range(HP):
            mm(accB[nb][:], x8v[kp], l1v[kp], nb * 128, stop=(kp == HP - 1))
        with nc.allow_low_precision(reason="bf16 output"):
            dve_copy(oB[nb][:, 0, :], accB[nb])
        wb_store(outB_d[nb], oB[nb][:], 1)

        # final token block split into two 256-col groups; group 0 goes out
        # via HWDGE (sync), group 1 via writeback, so the two completion
        # chains overlap instead of serializing on one dispatch path
        nb = NB - 1
        for i, (lo, hi) in enumerate(((0, 256), (256, 512))):
            fin = ps.tile([128, hi - lo], f32, tag="ps", name=f"fin{i}")
            lora_mm(fin[:], nb, 1, lo, hi, start=True)
            for pi, (xs, ws) in enumerate(((x8v, w1v), (xlv, w1v), (x8v, l1v))):
                for kp in range(HP):
                    mm(fin[:], xs[kp], ws[kp], nb * 128, lo, hi,
                       stop=(pi == 2 and kp == HP - 1))
            if i == 0:
                with nc.allow_low_precision(reason="bf16 output"):
                    act_copy(oF0[:], fin)
                nc.sync.dma_start(outF0_d[:], oF0[:])
            else:
                with nc.allow_low_precision(reason="bf16 output"):
                    dve_copy(oF1[:, 0, :], fin)
                wb_store(outF1_d, oF1[:], 1)

    nc.compile()
    return nc


def _get_nc():
    if "nc" not in _CACHE:
        _CACHE["nc"] = _build()
    return _CACHE["nc"]


def _q8(a):
    return np.ascontiguousarray(a).astype(E4)


def _prep_in_maps(x, weight, gate_w, lora_U, lora_V):
    xt = np.ascontiguousarray(x.reshape(N_TOK, D_IN).T)      # (D_IN, N_TOK) f32
    x8 = xt.astype(E4)
    xl8 = (xt - x8.astype(np.float32)).astype(E4)

    wTs = np.ascontiguousarray(weight.T) * SW                # (D_IN, D_OUT)
    w8 = wTs.astype(E4)
    wl8 = (wTs - w8.astype(np.float32)).astype(E4)

    u2T = np.ascontiguousarray(lora_U.reshape(ER, D_IN).T) * SW
    gwT = np.ascontiguousarray(gate_w.T) * SW
    u2p = u2T.reshape(HB, 128, ER).transpose(1, 0, 2).reshape(128, HB * ER)
    gwp = np.concatenate([gwT, np.zeros_like(gwT)], axis=1)
    gwp = gwp.reshape(HB, 128, 2 * E).transpose(1, 0, 2).reshape(128, HB * 2 * E)
    v2 = np.ascontiguousarray(lora_V.transpose(0, 2, 1).reshape(ER, D_OUT))

    sel = np.zeros((E, 2, ER), dtype=np.float32)
    sel[:, 0, :] = np.repeat(np.eye(E, dtype=np.float32), R, axis=0).T / 16.0

    idx = np.zeros((128, 8), np.int32)

    def kp_pack(mat, kp, cols):
        """[128, 2*len(cols)] block for k-pair kp of a (D_IN, C) matrix."""
        c0 = mat[2 * kp * 128:(2 * kp + 1) * 128, cols]
        c1 = mat[(2 * kp + 1) * 128:(2 * kp + 2) * 128, cols]
        return np.concatenate([c0, c1], axis=1)

    c0s, c1s = slice(0, 512), slice(512, 1024)
    common_seg = {}
    per_core_seg = []
    # segment payloads follow SEGS chunk order
    w0p = [kp_pack(w8.astype(np.float32), k, c0s) for k in range(4)]
    l0p = [kp_pack(wl8.astype(np.float32), k, c0s) for k in range(4)]
    w1p = [kp_pack(w8.astype(np.float32), k, c1s) for k in range(4)]
    l1p = [kp_pack(wl8.astype(np.float32), k, c1s) for k in range(4)]
    v2p = v2.astype(np.float32)  # [128, 1024]

    common = {
        "sel": _q8(sel.reshape(E, 2 * ER)),
        "idx": np.ascontiguousarray(idx),
        "W1a": _q8(np.concatenate([w1p[0], w1p[1], v2p], axis=1)),
        "W1b": _q8(np.concatenate([w1p[2], w1p[3]], axis=1)),
        "L1a": _q8(np.concatenate([l1p[0], l1p[1]], axis=1)),
        "L1b": _q8(np.concatenate([l1p[2], l1p[3]], axis=1)),
    }
    in_maps = []
    for c in range(N_CORES):
        ts = slice(c * TOK, (c + 1) * TOK)
        x8c = x8[:, ts].astype(np.float32)
        xlc = xl8[:, ts].astype(np.float32)
        x8p = [kp_pack(x8c, k, slice(0, TOK)) for k in range(4)]
        xlp = [kp_pack(xlc, k, slice(0, TOK)) for k in range(4)]
        m = dict(common)
        for k in range(3):
            m[f"P{k}"] = _q8(np.concatenate([x8p[k], w0p[k]], axis=1))
            m[f"Q{k}"] = _q8(np.concatenate([xlp[k], l0p[k]], axis=1))
        m["P3"] = _q8(np.concatenate(
            [x8p[3], w0p[3], gwp.astype(np.float32), u2p.astype(np.float32)], axis=1))
        m["Q3"] = _q8(np.concatenate([xlp[3], l0p[3]], axis=1))
        in_maps.append(m)
    return in_maps


def kernel(x, weight, gate_w, lora_U, lora_V):
    from concourse import bass_utils

    x = np.asarray(x, dtype=np.float32)
    weight = np.asarray(weight, dtype=np.float32)
    gate_w = np.asarray(gate_w, dtype=np.float32)
    lora_U = np.asarray(lora_U, dtype=np.float32)
    lora_V = np.asarray(lora_V, dtype=np.float32)

    nc = _get_nc()
    in_maps = _prep_in_maps(x, weight, gate_w, lora_U, lora_V)
    res = bass_utils.run_bass_kernel_spmd(nc, in_maps, core_ids=list(range(N_CORES)))
    out = np.empty((N_TOK, D_OUT), dtype=np.float32)
    for c in range(N_CORES):
        r = res.results[c]
        o = out[c * TOK:(c + 1) * TOK]
        o[:, 0:512] = np.asarray(r["outA"], dtype=np.float32)
        for i in range(NB - 1):
            o[i * 128:(i + 1) * 128, 512:1024] = np.asarray(
                r[f"outB{i}"], dtype=np.float32)
        o[384:512, 512:768] = np.asarray(r["outF0"], dtype=np.float32)
        o[384:512, 768:1024] = np.asarray(r["outF1"], dtype=np.float32)
    return out.reshape(B, S, D_OUT)


# revision 10
# speedup vs baseline: 1.1394x; 1.1394x over previous
"""MoELoRA forward on 8 Trainium2 NeuronCores — fp8 split-precision version.

Data-parallel over tokens (512/core). All heavy matmuls in fp8e4 with
DoubleRow perf mode (2 K-chunks of 128 per instruction at 0.5 cyc/row).
Precision: W and x are each split hi+lo into two e4m3 tensors
(x ~= x8 + xl, 32*W^T ~= W8 + Wl); the base product uses three DR passes
x8@W8 + x8@Wl + xl@W8 (the xl@Wl term is ~0.13% and dropped), giving
~bf16-level accuracy at 0.75x bf16's PE cost. The global x32 weight scale
dodges e4m3's subnormal floor (W sigma ~ 1/32) and is divided out in the
PSUM->SBUF output copies. The LoRA path (~5% of output magnitude) runs
pure fp8: gates 32-scaled into exp(scale=1/32), t = x8@(32 U2^T),
tp = e4(t_psum * gate/16), out += tp @ e4(V2) (zero-padded DR).

Measured CPU-sim accuracy of this exact scheme: ~2.9e-3 rel err (gate 2e-2).
"""

import numpy as np
import ml_dtypes

_CACHE = {}

B, S, D_IN, D_OUT, E, R = 4, 1024, 1024, 1024, 8, 16
N_CORES = 8
N_TOK = B * S
TOK = N_TOK // N_CORES        # 512 tokens/core
ER = E * R                    # 128
HB = D_IN // 128              # 8 k-chunks
HP = HB // 2                  # 4 k-pairs
NB = TOK // 128               # 4 token blocks
SW = 32.0                     # global weight scale

BF16 = ml_dtypes.bfloat16
E4 = ml_dtypes.float8_e4m3


def _build():
    import concourse.tile as tile
    import concourse.bass_isa as bass_isa
    from concourse import bacc, mybir
    from contextlib import ExitStack

    f32 = mybir.dt.float32
    bf16 = mybir.dt.bfloat16
    fp8 = mybir.dt.float8e4
    DR = mybir.MatmulPerfMode.DoubleRow
    Exp = mybir.ActivationFunctionType.Exp
    Copy = mybir.ActivationFunctionType.Copy

    nc = bacc.Bacc("TRN2", target_bir_lowering=False, debug=False,
                   num_devices=N_CORES)
    x8_d = nc.dram_tensor("x8", [D_IN, TOK], fp8, kind="ExternalInput").ap()
    xl_d = nc.dram_tensor("xl", [D_IN, TOK], fp8, kind="ExternalInput").ap()
    w8a0_d = nc.dram_tensor("w8a0", [D_IN // 2, 512], fp8, kind="ExternalInput").ap()
    w8a1_d = nc.dram_tensor("w8a1", [D_IN // 2, 512], fp8, kind="ExternalInput").ap()
    w8b_d = nc.dram_tensor("w8b", [D_IN, 512], fp8, kind="ExternalInput").ap()
    wla_d = nc.dram_tensor("wla", [D_IN, 512], fp8, kind="ExternalInput").ap()
    wlb_d = nc.dram_tensor("wlb", [D_IN, 512], fp8, kind="ExternalInput").ap()
    u28_d = nc.dram_tensor("u28", [128, HB * ER], fp8, kind="ExternalInput").ap()
    gw8_d = nc.dram_tensor("gw8", [128, HB * 2 * E], fp8, kind="ExternalInput").ap()
    v28_d = nc.dram_tensor("v28", [ER, D_OUT], fp8, kind="ExternalInput").ap()
    sel_d = nc.dram_tensor("sel", [E, 2 * ER], fp8, kind="ExternalInput").ap()
    out_d = nc.dram_tensor("out", [TOK, D_OUT], bf16, kind="ExternalOutput").ap()

    with tile.TileContext(nc) as tc, ExitStack() as ctx:
        sb = ctx.enter_context(tc.tile_pool(name="sb", bufs=1))
        ps = ctx.enter_context(tc.tile_pool(name="ps", bufs=8, space="PSUM"))
        osb = ctx.enter_context(tc.tile_pool(name="osb", bufs=4))

        x8 = sb.tile([128, HB, TOK], fp8, tag="x8")
        xl = sb.tile([128, HB, TOK], fp8, tag="xl")
        w8 = sb.tile([128, HB, D_OUT], fp8, tag="w8")
        wl = sb.tile([128, HB, D_OUT], fp8, tag="wl")
        u28 = sb.tile([128, HB, ER], fp8, tag="u28")
        gw8 = sb.tile([128, HB, 2 * E], fp8, tag="gw8")
        v28 = sb.tile([128, 2, D_OUT], fp8, tag="v28")
        selz = sb.tile([8, 2, ER], fp8, tag="selz")
        g8t = sb.tile([8, 2, TOK], fp8, tag="g8t")
        tpt = sb.tile([128, 2, TOK], fp8, tag="tpt")
        e_sb = sb.tile([8, TOK], f32, tag="e_sb")
        se_sb = sb.tile([8, TOK], f32, tag="se_sb")
        rec_sb = sb.tile([8, TOK], f32, tag="rec_sb")
        gp_sb = sb.tile([128, TOK], f32, tag="gp_sb")
        junk_sb = sb.tile([1, 512], bf16, tag="junk_sb")

        # junk first (on the otherwise-idle Pool engine) so the PE warmup
        # starts as early as possible; then zero the DR pair slots that never
        # get written (slot 1 of g8t/tpt)
        nc.gpsimd.memset(junk_sb[:], 0.0)
        nc.vector.memset(g8t[:], 0.0)
        nc.gpsimd.memset(tpt[:], 0.0)
        nc.gpsimd.memset(v28[:], 0.0)

        # ---- input DMAs (SP ring) in consumption order ----
        nc.sync.dma_start(x8[:], x8_d.rearrange("(hb p) n -> p hb n", p=128))
        nc.sync.dma_start(gw8[:], gw8_d[:])
        nc.sync.dma_start(w8[:, 0:4, 0:512],
                          w8a0_d.rearrange("(hb p) o -> p hb o", p=128))
        nc.sync.dma_start(w8[:, 4:8, 0:512],
                          w8a1_d.rearrange("(hb p) o -> p hb o", p=128))
        nc.sync.dma_start(xl[:], xl_d.rearrange("(hb p) n -> p hb n", p=128))
        nc.sync.dma_start(u28[:], u28_d[:])
        nc.sync.dma_start(selz[:], sel_d.rearrange("e (s r) -> e s r", s=2))
        nc.sync.dma_start(wl[:, :, 0:512], wla_d.rearrange("(hb p) o -> p hb o", p=128))
        nc.sync.dma_start(v28[:, 0, :], v28_d[:])
        nc.sync.dma_start(w8[:, :, 512:1024], w8b_d.rearrange("(hb p) o -> p hb o", p=128))
        nc.sync.dma_start(wl[:, :, 512:1024], wlb_d.rearrange("(hb p) o -> p hb o", p=128))

        # ---- PSUM banks: misc(warm+gl), t, accA0-3, junk2, gp; accB reuses ----
        misc = ps.tile([128, 512], f32, tag="ps", name="misc")
        t_ps = ps.tile([128, TOK], f32, tag="ps", name="t")
        accA = [ps.tile([128, 512], f32, tag="ps", name=f"accA{i}") for i in range(NB)]
        junk2 = ps.tile([1, 448], f32, tag="ps", name="junk2")

        def junk_fill(n):
            # keep the PE warm through a known DMA-supply gap
            for w in range(n):
                nc.tensor.matmul(junk2[:], junk_sb[0:1, 0:1], junk_sb[:, 0:448],
                                 start=(w == 0), stop=(w == n - 1),
                                 skip_group_check=True)

        # PE warmup during initial DMA dead-time
        N_WARM = 8
        nc.tensor.matmul(misc[0:1, 0:64], junk_sb[0:1, 0:1], junk_sb[:, 0:64],
                         start=True, stop=False, skip_group_check=True)
        for w in range(N_WARM):
            nc.tensor.matmul(misc[0:1, 0:448], junk_sb[0:1, 0:1], junk_sb[:, 0:448],
                             start=False, stop=(w == N_WARM - 1),
                             skip_group_check=True)

        def base_mm(acc, src_x, src_w, nb, oc, hp, start):
            nc.tensor.matmul(acc[nb][:],
                             src_x[:, 2 * hp:2 * hp + 2, nb * 128:(nb + 1) * 128],
                             src_w[:, 2 * hp:2 * hp + 2, oc * 512:(oc + 1) * 512],
                             start=start, stop=False,
                             perf_mode=DR, skip_group_check=True)

        def lora_mm(acc, nb, oc):
            nc.tensor.matmul(acc[nb][:],
                             tpt[:, :, nb * 128:(nb + 1) * 128],
                             v28[:, :, oc * 512:(oc + 1) * 512],
                             start=False, stop=True,
                             perf_mode=DR, skip_group_check=True)

        def store2(acc, nb0, oc, eng0, eng1, name):
            # non-final stores ride the Pool/SWDGE ring: no HWDGE contention
            # and the scalar engine's SEQ stays clear for copies
            o2 = osb.tile([128, 2, 512], bf16, tag="o", name=name)
            with nc.allow_low_precision(reason="bf16 output"):
                eng0(o2[:, 0, :], acc[nb0])
                eng1(o2[:, 1, :], acc[nb0 + 1])
            nc.gpsimd.dma_start(
                out_d.rearrange("(j p) o -> p j o", p=128)
                     [:, nb0:nb0 + 2, oc * 512:(oc + 1) * 512], o2[:])

        def dve_copy(dst, src):
            nc.vector.tensor_scalar(dst, src[:], 1.0 / SW, None,
                                    mybir.AluOpType.mult)

        def act_copy(dst, src):
            nc.scalar.activation(dst, src[:], Copy, scale=1.0 / SW)

        # gate logits gl[8, n] into misc rows 0:8 (DR pairs over k-chunks)
        for hp in range(HP):
            nc.tensor.matmul(misc[0:8, :], gw8[:, 2 * hp:2 * hp + 2, 0:E],
                             x8[:, 2 * hp:2 * hp + 2, :],
                             start=(hp == 0), stop=(hp == HP - 1),
                             perf_mode=DR, skip_group_check=True)
        # softmax chain: exp(gl/32) -> sum -> 1/sum -> g8 (fp8)
        nc.scalar.activation(e_sb[:], misc[0:8, :], Exp, scale=1.0 / SW)
        nc.gpsimd.partition_all_reduce(se_sb[:], e_sb[:], channels=8,
                                       reduce_op=bass_isa.ReduceOp.add)
        nc.vector.reciprocal(rec_sb[:], se_sb[:])
        with nc.allow_low_precision(reason="fp8 gate, lora path tolerance"):
            nc.vector.tensor_tensor(g8t[:, 0, :], e_sb[:], rec_sb[:],
                                    mybir.AluOpType.mult)

        # ---- phase A (oc=0); t-matmuls slotted between the W8 half-groups
        # to track the DMA arrival order (w8a0, u28, w8a1) ----
        oc = 0
        for hp in range(2):
            for nb in range(NB):
                base_mm(accA, x8, w8, nb, oc, hp, hp == 0)
        for hp in range(2, HP):
            for nb in range(NB):
                base_mm(accA, x8, w8, nb, oc, hp, False)
        for hp in range(HP):
            for nb in range(NB):
                base_mm(accA, xl, w8, nb, oc, hp, False)
        for hp in range(HP):
            nc.tensor.matmul(t_ps[:], u28[:, 2 * hp:2 * hp + 2, :],
                             x8[:, 2 * hp:2 * hp + 2, :],
                             start=(hp == 0), stop=(hp == HP - 1),
                             perf_mode=DR, skip_group_check=True)
        # gate expand: gp[er, n] = g8[er//16, n]/16 (zero-padded DR), then
        # gp -> SBUF (ACT), tp = t * gp -> fp8 (DVE)
        gp_ps = ps.tile([128, TOK], f32, tag="ps", name="gp")
        nc.tensor.matmul(gp_ps[:], selz[:], g8t[:], start=True, stop=True,
                         perf_mode=DR, skip_group_check=True)
        nc.scalar.copy(gp_sb[:], gp_ps[:])
        with nc.allow_low_precision(reason="fp8 tp, lora path tolerance"):
            nc.vector.tensor_tensor(tpt[:, 0, :], t_ps[:], gp_sb[:],
                                    mybir.AluOpType.mult)
        for hp in range(HP):
            for nb in range(NB):
                base_mm(accA, x8, wl, nb, oc, hp, False)
        for nb in range(NB):
            lora_mm(accA, nb, oc)
        store2(accA, 0, oc, dve_copy, act_copy, "oA01")
        store2(accA, 2, oc, dve_copy, act_copy, "oA23")

        # ---- phase B (oc=1), bank-major so the stops stagger and the
        # copy+store pipeline drains behind the PE instead of after it ----
        oc = 1
        oview = out_d.rearrange("(j p) o -> p j o", p=128)
        accB = [ps.tile([128, 512], f32, tag="ps", name=f"accB{i}")
                for i in range(NB - 1)]
        for nb in range(NB - 1):
            for hp in range(HP):
                base_mm(accB, x8, w8, nb, oc, hp, hp == 0)
            for hp in range(HP):
                base_mm(accB, xl, w8, nb, oc, hp, False)
            for hp in range(HP):
                base_mm(accB, x8, wl, nb, oc, hp, False)
            lora_mm(accB, nb, oc)
            o1 = osb.tile([128, 512], bf16, tag="o", name=f"oB{nb}")
            with nc.allow_low_precision(reason="bf16 output"):
                if nb % 2 == 0:
                    dve_copy(o1[:], accB[nb])
                else:
                    act_copy(o1[:], accB[nb])
            nc.gpsimd.dma_start(
                oview[:, nb:nb + 1, oc * 512:(oc + 1) * 512], o1[:])

        # final bank split by columns into two separate PSUM tiles (separate
        # tiles so group 2's matmuls don't serialize behind group 1's copy),
        # but sharing ONE staging tile and ONE store so the tail pays a single
        # HWDGE+DGE+sem chain
        nb = NB - 1
        o3 = osb.tile([128, 512], bf16, tag="of", name="oB3")
        for i, (lo, hi, eng, pnm) in enumerate(((0, 384, dve_copy, "fin0"),
                                                (384, 512, act_copy, "fin1"))):
            fin = ps.tile([128, hi - lo], f32, tag="ps", name=pnm)
            for hp in range(HP):
                nc.tensor.matmul(fin[:],
                                 x8[:, 2 * hp:2 * hp + 2, nb * 128:(nb + 1) * 128],
                                 w8[:, 2 * hp:2 * hp + 2, oc * 512 + lo:oc * 512 + hi],
                                 start=(hp == 0), stop=False,
                                 perf_mode=DR, skip_group_check=True)
            for src_x, src_w in ((xl, w8), (x8, wl)):
                for hp in range(HP):
                    nc.tensor.matmul(fin[:],
                                     src_x[:, 2 * hp:2 * hp + 2, nb * 128:(nb + 1) * 128],
                                     src_w[:, 2 * hp:2 * hp + 2, oc * 512 + lo:oc * 512 + hi],
                                     start=False, stop=False,
                                     perf_mode=DR, skip_group_check=True)
            nc.tensor.matmul(fin[:],
                             tpt[:, :, nb * 128:(nb + 1) * 128],
                             v28[:, :, oc * 512 + lo:oc * 512 + hi],
                             start=False, stop=True,
                             perf_mode=DR, skip_group_check=True)
            with nc.allow_low_precision(reason="bf16 output"):
                eng(o3[:, lo:hi], fin[:])
        nc.sync.dma_start(oview[:, nb:nb + 1, oc * 512:(oc + 1) * 512], o3[:])

    nc.compile()
    return nc


def _get_nc():
    if "nc" not in _CACHE:
        _CACHE["nc"] = _build()
    return _CACHE["nc"]


def _q8(a):
    return np.ascontiguousarray(a).astype(E4)


def _prep_in_maps(x, weight, gate_w, lora_U, lora_V):
    xt = np.ascontiguousarray(x.reshape(N_TOK, D_IN).T)      # (D_IN, N_TOK) f32
    x8 = xt.astype(E4)
    xlr = xt - x8.astype(np.float32)
    xl8 = xlr.astype(E4)

    wTs = np.ascontiguousarray(weight.T) * SW                # (D_IN, D_OUT)
    w8 = wTs.astype(E4)
    wl8 = (wTs - w8.astype(np.float32)).astype(E4)

    u2T = np.ascontiguousarray(lora_U.reshape(ER, D_IN).T) * SW
    gwT = np.ascontiguousarray(gate_w.T) * SW
    # pre-arrange the small operands into the SBUF per-partition layout so the
    # DMA reads large contiguous elements (full bus rate instead of min-time)
    u2T = u2T.reshape(HB, 128, ER).transpose(1, 0, 2).reshape(128, HB * ER)
    gwp = np.concatenate([gwT, np.zeros_like(gwT)], axis=1)
    gwp = gwp.reshape(HB, 128, 2 * E).transpose(1, 0, 2).reshape(128, HB * 2 * E)
    v2 = np.ascontiguousarray(lora_V.transpose(0, 2, 1).reshape(ER, D_OUT))

    sel = np.zeros((E, 2, ER), dtype=np.float32)
    sel[:, 0, :] = np.repeat(np.eye(E, dtype=np.float32), R, axis=0).T / 16.0

    common = {
        "w8a0": _q8(w8[0:512, 0:512]), "w8a1": _q8(w8[512:1024, 0:512]),
        "w8b": _q8(w8[:, 512:1024]),
        "wla": _q8(wl8[:, 0:512]), "wlb": _q8(wl8[:, 512:1024]),
        "u28": _q8(u2T),
        "gw8": _q8(gwp),
        "v28": _q8(v2),
        "sel": _q8(sel.reshape(E, 2 * ER)),
    }
    in_maps = []
    for c in range(N_CORES):
        m = dict(common)
        m["x8"] = np.ascontiguousarray(x8[:, c * TOK:(c + 1) * TOK])
        m["xl"] = np.ascontiguousarray(xl8[:, c * TOK:(c + 1) * TOK])
        in_maps.append(m)
    return in_maps


def kernel(x, weight, gate_w, lora_U, lora_V):
    from concourse import bass_utils

    x = np.asarray(x, dtype=np.float32)
    weight = np.asarray(weight, dtype=np.float32)
    gate_w = np.asarray(gate_w, dtype=np.float32)
    lora_U = np.asarray(lora_U, dtype=np.float32)
    lora_V = np.asarray(lora_V, dtype=np.float32)

    nc = _get_nc()
    in_maps = _prep_in_maps(x, weight, gate_w, lora_U, lora_V)
    res = bass_utils.run_bass_kernel_spmd(nc, in_maps, core_ids=list(range(N_CORES)))
    out = np.concatenate([np.asarray(res.results[c]["out"]) for c in range(N_CORES)],
                         axis=0)
    return out.astype(np.float32).reshape(B, S, D_OUT)



# revision 11
# speedup vs baseline: 1.1488x; 1.0083x over previous
"""MoELoRA forward on 8 Trainium2 NeuronCores — fp8 split-precision version.

Data-parallel over tokens (512/core). All heavy matmuls in fp8e4 with
DoubleRow perf mode (2 K-chunks of 128 per instruction at 0.5 cyc/row).
Precision: W and x are each split hi+lo into two e4m3 tensors
(x ~= x8 + xl, 32*W^T ~= W8 + Wl); the base product uses three DR passes
x8@W8 + x8@Wl + xl@W8 (the xl@Wl term is ~0.13% and dropped), giving
~bf16-level accuracy at 0.75x bf16's PE cost. The global x32 weight scale
dodges e4m3's subnormal floor (W sigma ~ 1/32) and is divided out in the
PSUM->SBUF output copies. The LoRA path (~5% of output magnitude) runs
pure fp8: gates 32-scaled into exp(scale=1/32), t = x8@(32 U2^T),
tp = e4(t_psum * gate/16), out += tp @ e4(V2) (zero-padded DR).

Measured CPU-sim accuracy of this exact scheme: ~2.9e-3 rel err (gate 2e-2).
"""

import numpy as np
import ml_dtypes

_CACHE = {}

B, S, D_IN, D_OUT, E, R = 4, 1024, 1024, 1024, 8, 16
N_CORES = 8
N_TOK = B * S
TOK = N_TOK // N_CORES        # 512 tokens/core
ER = E * R                    # 128
HB = D_IN // 128              # 8 k-chunks
HP = HB // 2                  # 4 k-pairs
NB = TOK // 128               # 4 token blocks
SW = 32.0                     # global weight scale

BF16 = ml_dtypes.bfloat16
E4 = ml_dtypes.float8_e4m3


def _build():
    import concourse.tile as tile
    import concourse.bass_isa as bass_isa
    from concourse import bacc, mybir
    from contextlib import ExitStack

    f32 = mybir.dt.float32
    bf16 = mybir.dt.bfloat16
    fp8 = mybir.dt.float8e4
    DR = mybir.MatmulPerfMode.DoubleRow
    Exp = mybir.ActivationFunctionType.Exp
    Copy = mybir.ActivationFunctionType.Copy

    nc = bacc.Bacc("TRN2", target_bir_lowering=False, debug=False,
                   num_devices=N_CORES)
    x8_d = nc.dram_tensor("x8", [D_IN, TOK], fp8, kind="ExternalInput").ap()
    xla_d = nc.dram_tensor("xla", [D_IN // 2, TOK], fp8, kind="ExternalInput").ap()
    xlb_d = nc.dram_tensor("xlb", [D_IN // 2, TOK], fp8, kind="ExternalInput").ap()
    w8a0_d = nc.dram_tensor("w8a0", [D_IN // 2, 512], fp8, kind="ExternalInput").ap()
    w8a1_d = nc.dram_tensor("w8a1", [D_IN // 2, 512], fp8, kind="ExternalInput").ap()
    w8b_d = nc.dram_tensor("w8b", [D_IN, 512], fp8, kind="ExternalInput").ap()
    wla_d = nc.dram_tensor("wla", [D_IN, 512], fp8, kind="ExternalInput").ap()
    wlb_d = nc.dram_tensor("wlb", [D_IN, 512], fp8, kind="ExternalInput").ap()
    u28_d = nc.dram_tensor("u28", [128, HB * ER], fp8, kind="ExternalInput").ap()
    gw8_d = nc.dram_tensor("gw8", [128, HB * 2 * E], fp8, kind="ExternalInput").ap()
    v28_d = nc.dram_tensor("v28", [ER, D_OUT], fp8, kind="ExternalInput").ap()
    sel_d = nc.dram_tensor("sel", [E, 2 * ER], fp8, kind="ExternalInput").ap()
    out_d = nc.dram_tensor("out", [TOK, D_OUT], bf16, kind="ExternalOutput").ap()

    with tile.TileContext(nc) as tc, ExitStack() as ctx:
        sb = ctx.enter_context(tc.tile_pool(name="sb", bufs=1))
        ps = ctx.enter_context(tc.tile_pool(name="ps", bufs=8, space="PSUM"))
        osb = ctx.enter_context(tc.tile_pool(name="osb", bufs=4))

        x8 = sb.tile([128, HB, TOK], fp8, tag="x8")
        xl = sb.tile([128, HB, TOK], fp8, tag="xl")
        w8 = sb.tile([128, HB, D_OUT], fp8, tag="w8")
        wl = sb.tile([128, HB, D_OUT], fp8, tag="wl")
        u28 = sb.tile([128, HB, ER], fp8, tag="u28")
        gw8 = sb.tile([128, HB, 2 * E], fp8, tag="gw8")
        v28 = sb.tile([128, 2, D_OUT], fp8, tag="v28")
        selz = sb.tile([8, 2, ER], fp8, tag="selz")
        g8t = sb.tile([8, 2, TOK], fp8, tag="g8t")
        tpt = sb.tile([128, 2, TOK], fp8, tag="tpt")
        e_sb = sb.tile([8, TOK], f32, tag="e_sb")
        se_sb = sb.tile([8, TOK], f32, tag="se_sb")
        rec_sb = sb.tile([8, TOK], f32, tag="rec_sb")
        gp_sb = sb.tile([128, TOK], f32, tag="gp_sb")
        junk_sb = sb.tile([1, 512], bf16, tag="junk_sb")

        # junk first (on the otherwise-idle Pool engine) so the PE warmup
        # starts as early as possible; then zero the DR pair slots that never
        # get written (slot 1 of g8t/tpt)
        nc.gpsimd.memset(junk_sb[:], 0.0)
        nc.vector.memset(g8t[:], 0.0)
        nc.gpsimd.memset(tpt[:], 0.0)
        nc.gpsimd.memset(v28[:], 0.0)

        # ---- input DMAs (SP ring) in consumption order ----
        nc.sync.dma_start(x8[:], x8_d.rearrange("(hb p) n -> p hb n", p=128))
        nc.sync.dma_start(gw8[:], gw8_d[:])
        nc.sync.dma_start(w8[:, 0:4, 0:512],
                          w8a0_d.rearrange("(hb p) o -> p hb o", p=128))
        nc.sync.dma_start(xl[:, 0:4, :],
                          xla_d.rearrange("(hb p) n -> p hb n", p=128))
        nc.sync.dma_start(w8[:, 4:8, 0:512],
                          w8a1_d.rearrange("(hb p) o -> p hb o", p=128))
        nc.sync.dma_start(xl[:, 4:8, :],
                          xlb_d.rearrange("(hb p) n -> p hb n", p=128))
        nc.sync.dma_start(u28[:], u28_d[:])
        nc.sync.dma_start(selz[:], sel_d.rearrange("e (s r) -> e s r", s=2))
        nc.sync.dma_start(wl[:, :, 0:512], wla_d.rearrange("(hb p) o -> p hb o", p=128))
        nc.sync.dma_start(v28[:, 0, :], v28_d[:])
        nc.sync.dma_start(w8[:, :, 512:1024], w8b_d.rearrange("(hb p) o -> p hb o", p=128))
        nc.sync.dma_start(wl[:, :, 512:1024], wlb_d.rearrange("(hb p) o -> p hb o", p=128))

        # ---- PSUM banks: misc(warm+gl), t, accA0-3, junk2, gp; accB reuses ----
        misc = ps.tile([128, 512], f32, tag="ps", name="misc")
        t_ps = ps.tile([128, TOK], f32, tag="ps", name="t")
        accA = [ps.tile([128, 512], f32, tag="ps", name=f"accA{i}") for i in range(NB)]
        junk2 = ps.tile([1, 448], f32, tag="ps", name="junk2")

        def junk_fill(n):
            # keep the PE warm through a known DMA-supply gap
            for w in range(n):
                nc.tensor.matmul(junk2[:], junk_sb[0:1, 0:1], junk_sb[:, 0:448],
                                 start=(w == 0), stop=(w == n - 1),
                                 skip_group_check=True)

        # PE warmup during initial DMA dead-time
        N_WARM = 8
        nc.tensor.matmul(misc[0:1, 0:64], junk_sb[0:1, 0:1], junk_sb[:, 0:64],
                         start=True, stop=False, skip_group_check=True)
        for w in range(N_WARM):
            nc.tensor.matmul(misc[0:1, 0:448], junk_sb[0:1, 0:1], junk_sb[:, 0:448],
                             start=False, stop=(w == N_WARM - 1),
                             skip_group_check=True)

        def base_mm(acc, src_x, src_w, nb, oc, hp, start):
            nc.tensor.matmul(acc[nb][:],
                             src_x[:, 2 * hp:2 * hp + 2, nb * 128:(nb + 1) * 128],
                             src_w[:, 2 * hp:2 * hp + 2, oc * 512:(oc + 1) * 512],
                             start=start, stop=False,
                             perf_mode=DR, skip_group_check=True)

        def lora_mm(acc, nb, oc):
            nc.tensor.matmul(acc[nb][:],
                             tpt[:, :, nb * 128:(nb + 1) * 128],
                             v28[:, :, oc * 512:(oc + 1) * 512],
                             start=False, stop=True,
                             perf_mode=DR, skip_group_check=True)

        def store2(acc, nb0, oc, eng0, eng1, name):
            # non-final stores ride the Pool/SWDGE ring: no HWDGE contention
            # and the scalar engine's SEQ stays clear for copies
            o2 = osb.tile([128, 2, 512], bf16, tag="o", name=name)
            with nc.allow_low_precision(reason="bf16 output"):
                eng0(o2[:, 0, :], acc[nb0])
                eng1(o2[:, 1, :], acc[nb0 + 1])
            nc.gpsimd.dma_start(
                out_d.rearrange("(j p) o -> p j o", p=128)
                     [:, nb0:nb0 + 2, oc * 512:(oc + 1) * 512], o2[:])

        def dve_copy(dst, src):
            nc.vector.tensor_scalar(dst, src[:], 1.0 / SW, None,
                                    mybir.AluOpType.mult)

        def act_copy(dst, src):
            nc.scalar.activation(dst, src[:], Copy, scale=1.0 / SW)

        # gate logits gl[8, n] into misc rows 0:8 (DR pairs over k-chunks)
        for hp in range(HP):
            nc.tensor.matmul(misc[0:8, :], gw8[:, 2 * hp:2 * hp + 2, 0:E],
                             x8[:, 2 * hp:2 * hp + 2, :],
                             start=(hp == 0), stop=(hp == HP - 1),
                             perf_mode=DR, skip_group_check=True)
        # softmax chain: exp(gl/32) -> sum -> 1/sum -> g8 (fp8)
        nc.scalar.activation(e_sb[:], misc[0:8, :], Exp, scale=1.0 / SW)
        nc.gpsimd.partition_all_reduce(se_sb[:], e_sb[:], channels=8,
                                       reduce_op=bass_isa.ReduceOp.add)
        nc.vector.reciprocal(rec_sb[:], se_sb[:])
        with nc.allow_low_precision(reason="fp8 gate, lora path tolerance"):
            nc.vector.tensor_tensor(g8t[:, 0, :], e_sb[:], rec_sb[:],
                                    mybir.AluOpType.mult)

        # ---- phase A (oc=0); t-matmuls slotted between the W8 half-groups
        # to track the DMA arrival order (w8a0, u28, w8a1) ----
        oc = 0
        for hp in range(2):
            for nb in range(NB):
                base_mm(accA, x8, w8, nb, oc, hp, hp == 0)
        for hp in range(2):
            for nb in range(NB):
                base_mm(accA, xl, w8, nb, oc, hp, False)
        for hp in range(2, HP):
            for nb in range(NB):
                base_mm(accA, x8, w8, nb, oc, hp, False)
        for hp in range(2, HP):
            for nb in range(NB):
                base_mm(accA, xl, w8, nb, oc, hp, False)
        for hp in range(HP):
            nc.tensor.matmul(t_ps[:], u28[:, 2 * hp:2 * hp + 2, :],
                             x8[:, 2 * hp:2 * hp + 2, :],
                             start=(hp == 0), stop=(hp == HP - 1),
                             perf_mode=DR, skip_group_check=True)
        # gate expand: gp[er, n] = g8[er//16, n]/16 (zero-padded DR), then
        # gp -> SBUF (ACT), tp = t * gp -> fp8 (DVE)
        gp_ps = ps.tile([128, TOK], f32, tag="ps", name="gp")
        nc.tensor.matmul(gp_ps[:], selz[:], g8t[:], start=True, stop=True,
                         perf_mode=DR, skip_group_check=True)
        nc.scalar.copy(gp_sb[:], gp_ps[:])
        with nc.allow_low_precision(reason="fp8 tp, lora path tolerance"):
            nc.vector.tensor_tensor(tpt[:, 0, :], t_ps[:], gp_sb[:],
                                    mybir.AluOpType.mult)
        for hp in range(HP):
            for nb in range(NB):
                base_mm(accA, x8, wl, nb, oc, hp, False)
        for nb in range(NB):
            lora_mm(accA, nb, oc)
        store2(accA, 0, oc, dve_copy, act_copy, "oA01")
        store2(accA, 2, oc, dve_copy, act_copy, "oA23")

        # ---- phase B (oc=1), bank-major so the stops stagger and the
        # copy+store pipeline drains behind the PE instead of after it ----
        oc = 1
        oview = out_d.rearrange("(j p) o -> p j o", p=128)
        accB = [ps.tile([128, 512], f32, tag="ps", name=f"accB{i}")
                for i in range(NB - 1)]
        for nb in range(NB - 1):
            for hp in range(HP):
                base_mm(accB, x8, w8, nb, oc, hp, hp == 0)
            for hp in range(HP):
                base_mm(accB, xl, w8, nb, oc, hp, False)
            for hp in range(HP):
                base_mm(accB, x8, wl, nb, oc, hp, False)
            lora_mm(accB, nb, oc)
            o1 = osb.tile([128, 512], bf16, tag="o", name=f"oB{nb}")
            with nc.allow_low_precision(reason="bf16 output"):
                if nb % 2 == 0:
                    dve_copy(o1[:], accB[nb])
                else:
                    act_copy(o1[:], accB[nb])
            nc.gpsimd.dma_start(
                oview[:, nb:nb + 1, oc * 512:(oc + 1) * 512], o1[:])

        # final bank split by columns into two separate PSUM tiles (separate
        # tiles so group 2's matmuls don't serialize behind group 1's copy),
        # but sharing ONE staging tile and ONE store so the tail pays a single
        # HWDGE+DGE+sem chain
        nb = NB - 1
        o3 = osb.tile([128, 512], bf16, tag="of", name="oB3")
        for i, (lo, hi, eng, pnm) in enumerate(((0, 384, dve_copy, "fin0"),
                                                (384, 512, act_copy, "fin1"))):
            fin = ps.tile([128, hi - lo], f32, tag="ps", name=pnm)
            for hp in range(HP):
                nc.tensor.matmul(fin[:],
                                 x8[:, 2 * hp:2 * hp + 2, nb * 128:(nb + 1) * 128],
                                 w8[:, 2 * hp:2 * hp + 2, oc * 512 + lo:oc * 512 + hi],
                                 start=(hp == 0), stop=False,
                                 perf_mode=DR, skip_group_check=True)
            for src_x, src_w in ((xl, w8), (x8, wl)):
                for hp in range(HP):
                    nc.tensor.matmul(fin[:],
                                     src_x[:, 2 * hp:2 * hp + 2, nb * 128:(nb + 1) * 128],
                                     src_w[:, 2 * hp:2 * hp + 2, oc * 512 + lo:oc * 512 + hi],
                                     start=False, stop=False,
                                     perf_mode=DR, skip_group_check=True)
            nc.tensor.matmul(fin[:],
                             tpt[:, :, nb * 128:(nb + 1) * 128],
                             v28[:, :, oc * 512 + lo:oc * 512 + hi],
                             start=False, stop=True,
                             perf_mode=DR, skip_group_check=True)
            with nc.allow_low_precision(reason="bf16 output"):
                eng(o3[:, lo:hi], fin[:])
        nc.sync.dma_start(oview[:, nb:nb + 1, oc * 512:(oc + 1) * 512], o3[:])

    nc.compile()
    return nc


def _get_nc():
    if "nc" not in _CACHE:
        _CACHE["nc"] = _build()
    return _CACHE["nc"]


def _q8(a):
    return np.ascontiguousarray(a).astype(E4)


def _prep_in_maps(x, weight, gate_w, lora_U, lora_V):
    xt = np.ascontiguousarray(x.reshape(N_TOK, D_IN).T)      # (D_IN, N_TOK) f32
    x8 = xt.astype(E4)
    xlr = xt - x8.astype(np.float32)
    xl8 = xlr.astype(E4)

    wTs = np.ascontiguousarray(weight.T) * SW                # (D_IN, D_OUT)
    w8 = wTs.astype(E4)
    wl8 = (wTs - w8.astype(np.float32)).astype(E4)

    u2T = np.ascontiguousarray(lora_U.reshape(ER, D_IN).T) * SW
    gwT = np.ascontiguousarray(gate_w.T) * SW
    # pre-arrange the small operands into the SBUF per-partition layout so the
    # DMA reads large contiguous elements (full bus rate instead of min-time)
    u2T = u2T.reshape(HB, 128, ER).transpose(1, 0, 2).reshape(128, HB * ER)
    gwp = np.concatenate([gwT, np.zeros_like(gwT)], axis=1)
    gwp = gwp.reshape(HB, 128, 2 * E).transpose(1, 0, 2).reshape(128, HB * 2 * E)
    v2 = np.ascontiguousarray(lora_V.transpose(0, 2, 1).reshape(ER, D_OUT))

    sel = np.zeros((E, 2, ER), dtype=np.float32)
    sel[:, 0, :] = np.repeat(np.eye(E, dtype=np.float32), R, axis=0).T / 16.0

    common = {
        "w8a0": _q8(w8[0:512, 0:512]), "w8a1": _q8(w8[512:1024, 0:512]),
        "w8b": _q8(w8[:, 512:1024]),
        "wla": _q8(wl8[:, 0:512]), "wlb": _q8(wl8[:, 512:1024]),
        "u28": _q8(u2T),
        "gw8": _q8(gwp),
        "v28": _q8(v2),
        "sel": _q8(sel.reshape(E, 2 * ER)),
    }
    in_maps = []
    for c in range(N_CORES):
        m = dict(common)
        m["x8"] = np.ascontiguousarray(x8[:, c * TOK:(c + 1) * TOK])
        m["xla"] = np.ascontiguousarray(xl8[0:512, c * TOK:(c + 1) * TOK])
        m["xlb"] = np.ascontiguousarray(xl8[512:1024, c * TOK:(c + 1) * TOK])
        in_maps.append(m)
    return in_maps


def kernel(x, weight, gate_w, lora_U, lora_V):
    from concourse import bass_utils

    x = np.asarray(x, dtype=np.float32)
    weight = np.asarray(weight, dtype=np.float32)
    gate_w = np.asarray(gate_w, dtype=np.float32)
    lora_U = np.asarray(lora_U, dtype=np.float32)
    lora_V = np.asarray(lora_V, dtype=np.float32)

    nc = _get_nc()
    in_maps = _prep_in_maps(x, weight, gate_w, lora_U, lora_V)
    res = bass_utils.run_bass_kernel_spmd(nc, in_maps, core_ids=list(range(N_CORES)))
    out = np.concatenate([np.asarray(res.results[c]["out"]) for c in range(N_CORES)],
                         axis=0)
    return out.astype(np.float32).reshape(B, S, D_OUT)

